# revision 33
# baseline (speedup 1.0000x reference)
"""BiasAttention TRN2 kernel — q-sharded across 8 NeuronCores.

Each core owns a block of 128 queries and computes the full attention for
them (all 8 heads, all 1024 keys), including the z-bias projection, with no
collectives.  The whole attention is computed TRANSPOSED (S^T[k,q],
bias^T[k,q]) so the exp'd scores feed the PV matmul directly with no PE
transposes.  Host-side prep lays z out per core as [kc, c, q, k] (contract
dim c on partitions, one query's [c,k] tile per stationary) and casts z to
fp8e4m3 — halving the dominant HBM stream.  Exact-math folds: bb and the
K-side bkv bias are constant along the softmax axis (cancel), and the
V-side bkv bias folds into bp on host (softmax rows sum to 1).

z streams as 1MB [c, 64q, 128k] transfers split across BOTH HWDGE rings so
the per-transfer completion latency overlaps and the stream runs near the
~360GB/s HBM-per-core limit.  All small consts ride in ONE per-partition-
contiguous packed transfer at the head of the scalar ring (separate
rearranged transfers produce 512B descriptors that crawl when fair-shared
against the z stream).  S^T / V builds are hoisted into the early z-ramp
window; the epilogue runs in bf16 (fp32 matmuls are 4 cyc/row).
"""

import sys

if "/opt/trn_rl_repo" not in sys.path:
    sys.path.insert(0, "/opt/trn_rl_repo")

import ml_dtypes
import numpy as np

import concourse.bass as bass
import concourse.mybir as mybir
from concourse import bacc
from concourse.bass_utils import run_bass_kernel_spmd
from concourse.masks import make_identity
from concourse.tile import TileContext

P = 128          # partitions
H = 8            # heads
D = 32           # head dim
CQ = 256         # q channels
CKV = 256        # kv channels
BD = 128         # bias (z) channels
NQ = 1024        # total queries
NCORES = 8
NQC = NQ // NCORES   # queries per core = 128
SCALE = D ** (-0.5)

QH = 64          # queries per z half-step (1MB fp8 transfer)
FP = mybir.dt.float32
BF = mybir.dt.bfloat16
F8 = mybir.dt.float8e4
NP_BF = ml_dtypes.bfloat16
NP_F8 = ml_dtypes.float8_e4m3

# bf16 const pack layout (elements per partition): wb, bq, Wq, xqT, Wkv,
# xkvT, Wp — all [p, ...] with the 2 c/hd-halves as the middle dim.
PK_WB = 0
PK_BQ = PK_WB + H
PK_WQ = PK_BQ + 2
PK_XQ = PK_WQ + 2 * CQ
PK_WKV = PK_XQ + 2 * NQC
PK_XKV = PK_WKV + 2 * 2 * H * D
PK_WP = PK_XKV + 2 * 1024
PK_END = PK_WP + 2 * CQ

# All z rides the sync HWDGE ring: only 8 HWDGE semaphore lanes exist
# (shared across both rings in scheduler order), so splitting z across
# rings couples their pacing and lets the Tile scheduler push the second
# ring's triggers behind the loop's exps.  One consumption-ordered ring
# sustains ~400GB/s by itself; the scalar ring carries only the const
# pack, and the kT32/qT32 shuffles use the gpsimd SWDGE (separate
# semaphore pool).
SCALAR_S = ()


def build_program(nk=1024):
    kc_n = nk // P            # k-chunks of 128
    ns = kc_n * 2             # z half-steps (64 queries x one k-chunk each)
    add = mybir.AluOpType.add
    mult = mybir.AluOpType.mult

    nc = bacc.Bacc("TRN2", target_bir_lowering=False, debug=False,
                   num_devices=NCORES)

    # ---- I/O ----
    zT = nc.dram_tensor("zT", [ns, BD, QH, P], F8, kind="ExternalInput")
    wbT = nc.dram_tensor("wbT", [BD, H], BF, kind="ExternalInput")
    packB = nc.dram_tensor("packB", [P, PK_END], BF, kind="ExternalInput")
    bp = nc.dram_tensor("bp", [CQ], FP, kind="ExternalInput")
    y = nc.dram_tensor("y", [NQC, CQ], FP, kind="ExternalOutput")

    with TileContext(nc) as tc:
        with (
            tc.tile_pool(name="const", bufs=1) as const,
            tc.tile_pool(name="zpool", bufs=15) as zpool,
            tc.tile_pool(name="epool", bufs=6) as epool,
            tc.tile_pool(name="xpool", bufs=5) as xpool,
            tc.tile_pool(name="stv_ps", bufs=2, space="PSUM") as stv_ps,
            tc.tile_pool(name="b_ps", bufs=5, space="PSUM") as b_psp,
            tc.tile_pool(name="o_ps", bufs=1, space="PSUM") as o_psp,
        ):
            # ---- const pack leads the scalar ring: one contiguous 8KB-per-
            # partition transfer (big descriptors survive fair-sharing with
            # the z stream).
            # wb leads as its own 2KB transfer: the very first bias matmuls
            # gate only on it plus the first z quarter.
            wb_sb = const.tile([P, H], BF)
            nc.scalar.dma_start(wb_sb, wbT[:])
            pk = const.tile([P, PK_END], BF)
            nc.scalar.dma_start(pk, packB[:])
            bq_sb = pk[:, PK_BQ:PK_WQ]
            wq_sb = pk[:, PK_WQ:PK_XQ].rearrange("p (o m) -> p o m", o=2)
            xqT_sb = pk[:, PK_XQ:PK_WKV].rearrange("p (o q) -> p o q", o=2)
            wkv_sb = pk[:, PK_WKV:PK_XKV].rearrange("p (o m) -> p o m", o=2)
            xkvT_sb = pk[:, PK_XKV:PK_WP].rearrange("p (o k) -> p o k", o=2)
            wp_sb = pk[:, PK_WP:PK_END].rearrange("p (o m) -> p o m", o=2)

            # ---- z stream: sync ring is pure z; the first half is split
            # into two 512KB transfers so the very first bias matmuls can
            # start sooner.
            zs = {}
            for s in range(ns):
                z_sb = zpool.tile([P, QH, P], F8, tag="z", name=f"z{s}")
                if s not in SCALAR_S:
                    if s == 0:
                        # 4x 256KB so the very first bias matmuls start early
                        for qq in range(4):
                            nc.sync.dma_start(z_sb[:, qq * 16:(qq + 1) * 16, :],
                                              zT[s, :, qq * 16:(qq + 1) * 16, :])
                    else:
                        nc.sync.dma_start(z_sb, zT[s])
                zs[s] = z_sb

            # bf16 cast during the SWDGE transfer (epilogue-only, tiny)
            bp_sb = const.tile([1, CQ], BF)
            nc.gpsimd.dma_start(bp_sb, bp[None, :])
            ident = const.tile([P, P], BF)
            make_identity(nc, ident)
            ones_row = const.tile([1, P], BF)
            nc.vector.memset(ones_row, 1.0)

            # V augmented with a ones column per head: [k, kc, h, D+1]
            vaug_sb = const.tile([P, kc_n, H, D + 1], BF)
            nc.vector.memset(vaug_sb, 1.0)

            # ---- bias^T z matmuls: per query a [c,k] stationary and an
            # 8-col Wb moving op into b_ps [k, (q64 h8)].  The S^T matmuls
            # then ACCUMULATE into the same psum (start=False, strided
            # columns q*8+h), so the logits are complete in PSUM and the
            # exp reads them directly — no DVE adds, no sT staging.
            bias_done = {}

            def bias_mms(b_ps, s, t0, t1):
                z_sb = zs[s]
                for t in range(t0, t1):
                    nc.tensor.matmul(b_ps[:, t * H:(t + 1) * H],
                                     lhsT=z_sb[:, t, :], rhs=wb_sb,
                                     start=(t == 0), stop=False)

            def st_mms(s):
                # S^T for this half accumulated on top of the bias
                kc, hf = s // 2, s % 2
                b_ps = bias_done[s]
                bv = b_ps.rearrange("p (q h) -> p q h", h=H)
                for h in range(H):
                    nc.tensor.matmul(bv[:, :, h],
                                     lhsT=kT32[:, h, kc * P:(kc + 1) * P],
                                     rhs=qT32[:, h, hf * QH:(hf + 1) * QH],
                                     start=False, stop=(h == H - 1))

            def bias_half(s):
                if s in bias_done:
                    return bias_done[s]
                b_ps = b_psp.tile([P, QH * H], FP, tag="b")
                bias_mms(b_ps, s, 0, QH)
                bias_done[s] = b_ps
                if s >= 4:
                    st_mms(s)
                return b_ps

            # hoisted startup: interleave the first z halves with the
            # projections in z-arrival order.
            b_ps0 = b_psp.tile([P, QH * H], FP, tag="b")
            bias_done[0] = b_ps0
            bias_mms(b_ps0, 0, 0, 16)        # s0 first quarter
            bias_mms(b_ps0, 0, 16, 32)

            # Q^T [(h d), q] with (x + bq) * SCALE folded in, stored bf16
            bq_f32 = const.tile([P, 2], FP)
            nc.vector.tensor_copy(bq_f32, bq_sb)
            qT_sb = const.tile([P, 2, NQC], BF)
            for m in range(2):
                ps = stv_ps.tile([P, 512], FP, tag="stv")
                for c in range(2):
                    nc.tensor.matmul(ps[:, :NQC],
                                     lhsT=wq_sb[:, c, m * P:(m + 1) * P],
                                     rhs=xqT_sb[:, c, :],
                                     start=(c == 0), stop=(c == 1))
                nc.vector.tensor_scalar(qT_sb[:, m, :], ps[:, :NQC],
                                        bq_f32[:, m:m + 1], SCALE, add, mult)

            bias_mms(b_ps0, 0, 32, QH)       # s0b

            # K^T [(h d), k] — bkv_K is softmax-invariant, dropped
            kT_sb = const.tile([P, 2, nk], BF)
            for m in range(2):
                for nh in range((nk + 511) // 512):
                    nn_ = min(512, nk - nh * 512)
                    ps = stv_ps.tile([P, 512], FP, tag="stv")
                    for c in range(2):
                        nc.tensor.matmul(ps[:, :nn_],
                                         lhsT=wkv_sb[:, c, m * P:(m + 1) * P],
                                         rhs=xkvT_sb[:, c, nh * 512:nh * 512 + nn_],
                                         start=(c == 0), stop=(c == 1))
                    if (m * 2 + nh) % 2 == 0:
                        nc.scalar.activation(
                            kT_sb[:, m, nh * 512:nh * 512 + nn_], ps[:, :nn_],
                            mybir.ActivationFunctionType.Copy)
                    else:
                        nc.vector.tensor_copy(
                            kT_sb[:, m, nh * 512:nh * 512 + nn_], ps[:, :nn_])

            bias_half(1)
            bias_half(2)
            bias_half(3)

            # qT/kT reshuffled so every head's d-dim sits on partitions
            # 0-31: the S^T matmuls then run at default tile position and
            # can share psum banks at different column offsets (the same
            # pattern the z matmuls use) — tile-positioned matmuls sharing
            # a bank wedge the device.  kT32 rides the scalar ring between
            # the early and late z halves; qT32 rides gpsimd.
            qT32 = const.tile([32, H, NQC], BF)
            kT32 = const.tile([32, H, nk], BF)
            for hq in range(4):
                nc.gpsimd.dma_start(qT32[:, hq::4, :],
                                    qT_sb[hq * 32:(hq + 1) * 32, :, :])
            for m in range(2):
                for hq in range(4):
                    nc.gpsimd.dma_start(kT32[:, m * 4 + hq, :],
                                        kT_sb[hq * 32:(hq + 1) * 32, m, :])

            # V [k, (h d)] (bias folded into bp on host) into vaug
            def v_build(kc):
                ps = stv_ps.tile([P, 512], FP, tag="stv", name="v_ps")
                for c in range(2):
                    nc.tensor.matmul(ps[:, :H * D],
                                     lhsT=xkvT_sb[:, c, kc * P:(kc + 1) * P],
                                     rhs=wkv_sb[:, c, H * D:2 * H * D],
                                     start=(c == 0), stop=(c == 1))
                nc.vector.tensor_copy(
                    vaug_sb[:, kc, :, 0:D],
                    ps[:, :H * D].rearrange("p (h d) -> p h d", h=H))

            # V builds run up front, then the deferred S^T accumulations
            # for the hoisted bias halves (they wait for kT32/qT32).
            for kc in range(kc_n):
                v_build(kc)
            for s in range(4):
                st_mms(s)

            # ---- main loop over k-chunks: pure bias+PV on the PE, with
            # the add->exp chain two chunks behind the z matmuls.
            o_ps = o_psp.tile([P, H * (D + 1)], FP)   # [q, h*(D+1)]
            xTs = {}

            def pv_emit(kc):
                xT_sb = xTs.pop(kc)
                for h in range(H):
                    # o_ps lives in one bank: open the accumulation group
                    # on the first matmul only, close on the last.
                    nc.tensor.matmul(
                        o_ps[:, h * (D + 1):(h + 1) * (D + 1)],
                        lhsT=xT_sb[:, h, :], rhs=vaug_sb[:, kc, h, :],
                        start=(kc == 0 and h == 0),
                        stop=(kc == kc_n - 1 and h == H - 1))

            for kc in range(kc_n):
                bps = [bias_half(2 * kc), bias_half(2 * kc + 1)]
                if kc - 3 in xTs:
                    pv_emit(kc - 3)
                if kc == kc_n - 1:
                    # exps for kc-2 finished long ago; drain it before the
                    # final chunk's own chain
                    pv_emit(kc - 2)
                # exp straight from psum: in (q64,h8)-major, out [k, h, q]
                xT_sb = xpool.tile([P, H, NQC], BF, tag="x")
                xTs[kc] = xT_sb
                for hf in range(2):
                    qs = slice(hf * QH, (hf + 1) * QH)
                    nc.scalar.activation(
                        xT_sb[:, :, qs].rearrange("p h q -> p q h"),
                        bps[hf], mybir.ActivationFunctionType.Exp)
            pv_emit(kc_n - 2)
            pv_emit(kc_n - 1)

            # ---- epilogue: normalize, transpose, output projection, all
            # in bf16 (fp32 matmuls run at 1/4 rate).
            recip_sb = const.tile([P, H], FP)
            nc.vector.reciprocal(
                recip_sb, o_ps.rearrange("p (h e) -> p h e", h=H)[:, :, D])
            o_sb = const.tile([P, 2, P], BF)     # [q, half, (h d)%128]
            ov = o_ps.rearrange("p (h e) -> p h e", h=H)
            rv = bass.AP(tensor=recip_sb.tensor, offset=recip_sb.offset,
                         ap=[list(recip_sb.ap[0])]
                         + [[recip_sb.ap[1][0], 4], [0, D]])
            for half in range(2):
                nc.vector.tensor_tensor(
                    o_sb[:, half, :].rearrange("p (h d) -> p h d", h=4),
                    ov[:, half * 4:(half + 1) * 4, 0:D],
                    bass.AP(tensor=rv.tensor,
                            offset=rv.offset + half * 4 * recip_sb.ap[1][0],
                            ap=rv.ap), mult)
            oT_sb = const.tile([P, 2, P], BF)
            for m in range(2):
                t_full = stv_ps.tile([P, 1024], BF, tag="stv", name="t_full")
                t_ps = t_full[:, :P]
                nc.tensor.transpose(t_ps, o_sb[:, m, :], ident)
                nc.vector.tensor_copy(oT_sb[:, m, :], t_ps)
            ps = stv_ps.tile([P, 512], FP, tag="stv")
            for m in range(2):
                nc.tensor.matmul(ps[:, :CQ], lhsT=oT_sb[:, m, :],
                                 rhs=wp_sb[:, m, :], start=(m == 0), stop=False)
            nc.tensor.matmul(ps[:, :CQ], lhsT=ones_row, rhs=bp_sb,
                             start=False, stop=True)
            y_sb = const.tile([P, CQ], FP)
            nc.vector.tensor_copy(y_sb, ps[:, :CQ])
            nc.sync.dma_start(y[:], y_sb)

    nc.compile()
    return nc


def prep_inputs(x_q, x_kv, z, Wq, bq, Wkv, bkv, Wb, bb, Wp, bp, nk=1024):
    """Host-side shard prep.  Returns in_maps for the 8 cores."""
    kc_n = nk // P
    # bkv_V folds into bp exactly: softmax rows sum to 1, so the +bkvV on V
    # adds bkvV @ Wp to every output row.  bb / bkv_K cancel in softmax.
    bp_eff = (np.asarray(bp, dtype=np.float64)
              + np.asarray(bkv[H * D:], dtype=np.float64)
              @ np.asarray(Wp, dtype=np.float64)).astype(np.float32)

    def halves(a2d):        # [2*P rows, m] -> [P, 2, m] -> [P, 2*m]
        r = np.asarray(a2d)
        return (r.reshape(2, P, -1).transpose(1, 0, 2)
                .reshape(P, -1).astype(NP_BF))

    wb_bf = np.ascontiguousarray(Wb).astype(NP_BF)
    pk_shared = np.empty((P, PK_END), dtype=NP_BF)
    pk_shared[:, PK_WB:PK_BQ] = wb_bf
    pk_shared[:, PK_BQ:PK_WQ] = (np.asarray(bq).reshape(2, P)
                                 .transpose(1, 0).astype(NP_BF))
    pk_shared[:, PK_WQ:PK_XQ] = halves(Wq)
    pk_shared[:, PK_WKV:PK_XKV] = halves(Wkv)
    pk_shared[:, PK_XKV:PK_WP] = halves(x_kv[0].T)
    pk_shared[:, PK_WP:PK_END] = halves(Wp)
    in_maps = []
    for i in range(NCORES):
        qs = i * NQC
        zi = z[0, qs:qs + NQC]                           # [q, k, c]
        # -> [s=(kc,qh), c, q64, k] with q = qh*64 + t, k = kc*128 + k
        zi = (zi.reshape(2, QH, kc_n, P, BD)
              .transpose(2, 0, 4, 1, 3)                  # [kc, qh, c, t, k]
              .reshape(kc_n * 2, BD, QH, P))
        pk = pk_shared.copy()
        pk[:, PK_XQ:PK_WKV] = halves(x_q[0, qs:qs + NQC].T)
        in_maps.append(dict(
            zT=np.ascontiguousarray(zi).astype(NP_F8),
            wbT=wb_bf,
            packB=pk,
            bp=bp_eff,
        ))
    return in_maps


_NC_CACHE = {}


def kernel(x_q, x_kv, z, Wq, bq, Wkv, bkv, Wb, bb, Wp, bp):
    key = "full"
    if key not in _NC_CACHE:
        _NC_CACHE[key] = build_program()
    nc = _NC_CACHE[key]
    in_maps = prep_inputs(x_q, x_kv, z, Wq, bq, Wkv, bkv, Wb, bb, Wp, bp)
    res = run_bass_kernel_spmd(nc, in_maps, list(range(NCORES)))
    out = np.empty((1, NQ, CQ), dtype=np.float32)
    for i in range(NCORES):
        out[0, i * NQC:(i + 1) * NQC, :] = res.results[i]["y"]
    return out


# revision 34
# speedup vs baseline: 1.1231x; 1.1231x over previous
"""BiasAttention TRN2 kernel — q-sharded across 8 NeuronCores.

Each core owns a block of 128 queries and computes the full attention for
them (all 8 heads, all 1024 keys), including the z-bias projection, with no
collectives.  The whole attention is computed TRANSPOSED (S^T[k,q],
bias^T[k,q]) so the exp'd scores feed the PV matmul directly with no PE
transposes.  Host-side prep lays z out per core as [kc, c, q, k] (contract
dim c on partitions, one query's [c,k] tile per stationary) and casts z to
fp8e4m3 — halving the dominant HBM stream.  Exact-math folds: bb and the
K-side bkv bias are constant along the softmax axis (cancel), and the
V-side bkv bias folds into bp on host (softmax rows sum to 1).

z streams as 1MB [c, 64q, 128k] transfers split across BOTH HWDGE rings so
the per-transfer completion latency overlaps and the stream runs near the
~360GB/s HBM-per-core limit.  All small consts ride in ONE per-partition-
contiguous packed transfer at the head of the scalar ring (separate
rearranged transfers produce 512B descriptors that crawl when fair-shared
against the z stream).  S^T / V builds are hoisted into the early z-ramp
window; the epilogue runs in bf16 (fp32 matmuls are 4 cyc/row).
"""

import sys

if "/opt/trn_rl_repo" not in sys.path:
    sys.path.insert(0, "/opt/trn_rl_repo")

import ml_dtypes
import numpy as np

import concourse.bass as bass
import concourse.mybir as mybir
from concourse import bacc
from concourse.bass_utils import run_bass_kernel_spmd
from concourse.masks import make_identity
from concourse.tile import TileContext

P = 128          # partitions
H = 8            # heads
D = 32           # head dim
CQ = 256         # q channels
CKV = 256        # kv channels
BD = 128         # bias (z) channels
NQ = 1024        # total queries
NCORES = 8
NQC = NQ // NCORES   # queries per core = 128
SCALE = D ** (-0.5)

QH = 64          # queries per z half-step (1MB fp8 transfer)
FP = mybir.dt.float32
BF = mybir.dt.bfloat16
F8 = mybir.dt.float8e4
NP_BF = ml_dtypes.bfloat16
NP_F8 = ml_dtypes.float8_e4m3

# bf16 const pack layout (elements per partition): wb, bq, Wq, xqT, Wkv,
# xkvT, Wp — all [p, ...] with the 2 c/hd-halves as the middle dim.
PK_WB = 0
PK_BQ = PK_WB + H
PK_WQ = PK_BQ + 2
PK_XQ = PK_WQ + 2 * CQ
PK_WKV = PK_XQ + 2 * NQC
PK_XKV = PK_WKV + 2 * 2 * H * D
PK_WP = PK_XKV + 2 * 1024
PK_END = PK_WP + 2 * CQ

# All z rides the sync HWDGE ring: only 8 HWDGE semaphore lanes exist
# (shared across both rings in scheduler order), so splitting z across
# rings couples their pacing and lets the Tile scheduler push the second
# ring's triggers behind the loop's exps.  One consumption-ordered ring
# sustains ~400GB/s by itself; the scalar ring carries only the const
# pack, and the kT32/qT32 shuffles use the gpsimd SWDGE (separate
# semaphore pool).
SCALAR_S = ()


def build_program(nk=1024):
    kc_n = nk // P            # k-chunks of 128
    ns = kc_n * 2             # z half-steps (64 queries x one k-chunk each)
    add = mybir.AluOpType.add
    mult = mybir.AluOpType.mult

    nc = bacc.Bacc("TRN2", target_bir_lowering=False, debug=False,
                   num_devices=NCORES)

    # ---- I/O ----
    zT = nc.dram_tensor("zT", [ns, BD, QH, P], F8, kind="ExternalInput")
    wbT = nc.dram_tensor("wbT", [BD, H], BF, kind="ExternalInput")
    packB = nc.dram_tensor("packB", [P, PK_END], BF, kind="ExternalInput")
    bp = nc.dram_tensor("bp", [CQ], FP, kind="ExternalInput")
    y = nc.dram_tensor("y", [NQC, CQ], FP, kind="ExternalOutput")

    with TileContext(nc) as tc:
        with (
            tc.tile_pool(name="const", bufs=1) as const,
            tc.tile_pool(name="zpool", bufs=15) as zpool,
            tc.tile_pool(name="epool", bufs=6) as epool,
            tc.tile_pool(name="xpool", bufs=5) as xpool,
            tc.tile_pool(name="stv_ps", bufs=2, space="PSUM") as stv_ps,
            tc.tile_pool(name="b_ps", bufs=5, space="PSUM") as b_psp,
            tc.tile_pool(name="o_ps", bufs=1, space="PSUM") as o_psp,
        ):
            # ---- const pack leads the scalar ring: one contiguous 8KB-per-
            # partition transfer (big descriptors survive fair-sharing with
            # the z stream).
            # wb leads as its own 2KB transfer: the very first bias matmuls
            # gate only on it plus the first z quarter.
            wb_sb = const.tile([P, H], BF)
            nc.scalar.dma_start(wb_sb, wbT[:])
            pk = const.tile([P, PK_END], BF)
            nc.scalar.dma_start(pk, packB[:])
            bq_sb = pk[:, PK_BQ:PK_WQ]
            wq_sb = pk[:, PK_WQ:PK_XQ].rearrange("p (o m) -> p o m", o=2)
            xqT_sb = pk[:, PK_XQ:PK_WKV].rearrange("p (o q) -> p o q", o=2)
            wkv_sb = pk[:, PK_WKV:PK_XKV].rearrange("p (o m) -> p o m", o=2)
            xkvT_sb = pk[:, PK_XKV:PK_WP].rearrange("p (o k) -> p o k", o=2)
            wp_sb = pk[:, PK_WP:PK_END].rearrange("p (o m) -> p o m", o=2)

            # ---- z stream: sync ring is pure z; the first half is split
            # into two 512KB transfers so the very first bias matmuls can
            # start sooner.
            zs = {}
            for s in range(ns):
                z_sb = zpool.tile([P, QH, P], F8, tag="z", name=f"z{s}")
                if s not in SCALAR_S:
                    if s == 0:
                        # 4x 256KB so the very first bias matmuls start early
                        for qq in range(4):
                            nc.sync.dma_start(z_sb[:, qq * 16:(qq + 1) * 16, :],
                                              zT[s, :, qq * 16:(qq + 1) * 16, :])
                    else:
                        nc.sync.dma_start(z_sb, zT[s])
                zs[s] = z_sb

            # bf16 cast during the SWDGE transfer (epilogue-only, tiny)
            bp_sb = const.tile([1, CQ], BF)
            nc.gpsimd.dma_start(bp_sb, bp[None, :])
            ident = const.tile([P, P], BF)
            make_identity(nc, ident)
            ones_row = const.tile([1, P], BF)
            nc.vector.memset(ones_row, 1.0)

            # V augmented with a ones column per head: [k, kc, h, D+1]
            vaug_sb = const.tile([P, kc_n, H, D + 1], BF)
            nc.vector.memset(vaug_sb, 1.0)

            # ---- bias^T z matmuls: per query a [c,k] stationary and an
            # 8-col Wb moving op into b_ps [k, (q64 h8)].
            bias_done = {}

            def bias_mms(b_ps, s, t0, t1):
                z_sb = zs[s]
                for t in range(t0, t1):
                    nc.tensor.matmul(b_ps[:, t * H:(t + 1) * H],
                                     lhsT=z_sb[:, t, :], rhs=wb_sb,
                                     start=(t == 0), stop=(t == QH - 1))

            def bias_half(s):
                if s in bias_done:
                    return bias_done[s]
                b_ps = b_psp.tile([P, QH * H], FP, tag="b")
                bias_mms(b_ps, s, 0, QH)
                bias_done[s] = b_ps
                return b_ps

            # hoisted startup: interleave the first z halves with the
            # projections in z-arrival order.
            b_ps0 = b_psp.tile([P, QH * H], FP, tag="b")
            bias_done[0] = b_ps0
            bias_mms(b_ps0, 0, 0, 16)        # s0 first quarter
            bias_mms(b_ps0, 0, 16, 32)

            # Q^T [(h d), q] with (x + bq) * SCALE folded in, stored bf16
            bq_f32 = const.tile([P, 2], FP)
            nc.vector.tensor_copy(bq_f32, bq_sb)
            qT_sb = const.tile([P, 2, NQC], BF)
            for m in range(2):
                ps = stv_ps.tile([P, 512], FP, tag="stv")
                for c in range(2):
                    nc.tensor.matmul(ps[:, :NQC],
                                     lhsT=wq_sb[:, c, m * P:(m + 1) * P],
                                     rhs=xqT_sb[:, c, :],
                                     start=(c == 0), stop=(c == 1))
                nc.vector.tensor_scalar(qT_sb[:, m, :], ps[:, :NQC],
                                        bq_f32[:, m:m + 1], SCALE, add, mult)

            bias_mms(b_ps0, 0, 32, QH)       # s0b

            # K^T [(h d), k] — bkv_K is softmax-invariant, dropped
            kT_sb = const.tile([P, 2, nk], BF)
            for m in range(2):
                for nh in range((nk + 511) // 512):
                    nn_ = min(512, nk - nh * 512)
                    ps = stv_ps.tile([P, 512], FP, tag="stv")
                    for c in range(2):
                        nc.tensor.matmul(ps[:, :nn_],
                                         lhsT=wkv_sb[:, c, m * P:(m + 1) * P],
                                         rhs=xkvT_sb[:, c, nh * 512:nh * 512 + nn_],
                                         start=(c == 0), stop=(c == 1))
                    if (m * 2 + nh) % 2 == 0:
                        nc.scalar.activation(
                            kT_sb[:, m, nh * 512:nh * 512 + nn_], ps[:, :nn_],
                            mybir.ActivationFunctionType.Copy)
                    else:
                        nc.vector.tensor_copy(
                            kT_sb[:, m, nh * 512:nh * 512 + nn_], ps[:, :nn_])

            bias_half(1)
            bias_half(2)

            # qT/kT reshuffled so every head's d-dim sits on partitions
            # 0-31: the S^T matmuls then run at default tile position and
            # can share psum banks at different column offsets (the same
            # pattern the z matmuls use) — tile-positioned matmuls sharing
            # a bank wedge the device.  kT32 rides the scalar ring between
            # the early and late z halves; qT32 rides gpsimd.
            qT32 = const.tile([32, H, NQC], BF)
            kT32 = const.tile([32, H, nk], BF)
            for hq in range(4):
                nc.gpsimd.dma_start(qT32[:, hq::4, :],
                                    qT_sb[hq * 32:(hq + 1) * 32, :, :])
            for m in range(2):
                for hq in range(4):
                    nc.gpsimd.dma_start(kT32[:, m * 4 + hq, :],
                                        kT_sb[hq * 32:(hq + 1) * 32, m, :])

            # S^T[k, kc, h, q]: per (kc, head) a [k=128, q=128] matmul;
            # 4 heads batch into one psum tile with a single copy out.
            sT_sb = const.tile([P, kc_n, H, NQC], BF)

            def st_build(kc):
                for hg in range(2):
                    ps = stv_ps.tile([P, 512], FP, tag="stv", name="st_ps")
                    for hl in range(4):
                        h = hg * 4 + hl
                        nc.tensor.matmul(ps[:, hl * P:(hl + 1) * P],
                                         lhsT=kT32[:, h, kc * P:(kc + 1) * P],
                                         rhs=qT32[:, h, :],
                                         start=True, stop=True)
                    nc.vector.tensor_copy(
                        sT_sb[:, kc, hg * 4:(hg + 1) * 4, :],
                        ps.rearrange("p (hl q) -> p hl q", hl=4))

            # V [k, (h d)] (bias folded into bp on host) into vaug
            def v_build(kc):
                ps = stv_ps.tile([P, 512], FP, tag="stv", name="v_ps")
                for c in range(2):
                    nc.tensor.matmul(ps[:, :H * D],
                                     lhsT=xkvT_sb[:, c, kc * P:(kc + 1) * P],
                                     rhs=wkv_sb[:, c, H * D:2 * H * D],
                                     start=(c == 0), stop=(c == 1))
                nc.vector.tensor_copy(
                    vaug_sb[:, kc, :, 0:D],
                    ps[:, :H * D].rearrange("p (h d) -> p h d", h=H))

            # all S^T / V builds run up front, filling the early window
            # while the z stream is still ramping.
            for kc in range(kc_n):
                v_build(kc)
            for kc in range(kc_n):
                st_build(kc)

            # ---- main loop over k-chunks: pure bias+PV on the PE, with
            # the add->exp chain two chunks behind the z matmuls.
            o_ps = o_psp.tile([P, H * (D + 1)], FP)   # [q, h*(D+1)]
            xTs = {}

            def pv_emit(kc):
                xT_sb = xTs.pop(kc)
                for h in range(H):
                    # o_ps lives in one bank: open the accumulation group
                    # on the first matmul only, close on the last.
                    nc.tensor.matmul(
                        o_ps[:, h * (D + 1):(h + 1) * (D + 1)],
                        lhsT=xT_sb[:, h, :], rhs=vaug_sb[:, kc, h, :],
                        start=(kc == 0 and h == 0),
                        stop=(kc == kc_n - 1 and h == H - 1))

            for kc in range(kc_n):
                bps = [bias_half(2 * kc), bias_half(2 * kc + 1)]
                # e^T holds the chunk's logits [k, h, q] (bf16)
                e_sb = epool.tile([P, H, NQC], BF, tag="e")
                for hf in range(2):
                    bv = bps[hf].rearrange("p (q h) -> p h q", h=H)
                    qs = slice(hf * QH, (hf + 1) * QH)
                    for hg in range(2):
                        hh = slice(hg * 4, (hg + 1) * 4)
                        nc.vector.tensor_tensor(
                            e_sb[:, hh, qs], sT_sb[:, kc, hh, qs],
                            bv[:, hh, :], add)
                if kc - 3 in xTs:
                    pv_emit(kc - 3)
                if kc == kc_n - 1:
                    # exps for kc-2 finished long ago; drain it before the
                    # final chunk's own chain
                    pv_emit(kc - 2)
                xT_sb = xpool.tile([P, H, NQC], BF, tag="x")
                xTs[kc] = xT_sb
                for hg in range(2):          # head groups of 4
                    hh = slice(hg * 4, (hg + 1) * 4)
                    nc.scalar.activation(xT_sb[:, hh, :], e_sb[:, hh, :],
                                         mybir.ActivationFunctionType.Exp)
            pv_emit(kc_n - 2)
            pv_emit(kc_n - 1)

            # ---- epilogue: normalize, transpose, output projection, all
            # in bf16 (fp32 matmuls run at 1/4 rate).
            recip_sb = const.tile([P, H], FP)
            nc.vector.reciprocal(
                recip_sb, o_ps.rearrange("p (h e) -> p h e", h=H)[:, :, D])
            o_sb = const.tile([P, 2, P], BF)     # [q, half, (h d)%128]
            ov = o_ps.rearrange("p (h e) -> p h e", h=H)
            rv = bass.AP(tensor=recip_sb.tensor, offset=recip_sb.offset,
                         ap=[list(recip_sb.ap[0])]
                         + [[recip_sb.ap[1][0], 4], [0, D]])
            for half in range(2):
                nc.vector.tensor_tensor(
                    o_sb[:, half, :].rearrange("p (h d) -> p h d", h=4),
                    ov[:, half * 4:(half + 1) * 4, 0:D],
                    bass.AP(tensor=rv.tensor,
                            offset=rv.offset + half * 4 * recip_sb.ap[1][0],
                            ap=rv.ap), mult)
            oT_sb = const.tile([P, 2, P], BF)
            for m in range(2):
                t_full = stv_ps.tile([P, 1024], BF, tag="stv", name="t_full")
                t_ps = t_full[:, :P]
                nc.tensor.transpose(t_ps, o_sb[:, m, :], ident)
                nc.vector.tensor_copy(oT_sb[:, m, :], t_ps)
            ps = stv_ps.tile([P, 512], FP, tag="stv")
            for m in range(2):
                nc.tensor.matmul(ps[:, :CQ], lhsT=oT_sb[:, m, :],
                                 rhs=wp_sb[:, m, :], start=(m == 0), stop=False)
            nc.tensor.matmul(ps[:, :CQ], lhsT=ones_row, rhs=bp_sb,
                             start=False, stop=True)
            y_sb = const.tile([P, CQ], FP)
            nc.vector.tensor_copy(y_sb, ps[:, :CQ])
            nc.sync.dma_start(y[:], y_sb)

    nc.compile()
    return nc


def prep_inputs(x_q, x_kv, z, Wq, bq, Wkv, bkv, Wb, bb, Wp, bp, nk=1024):
    """Host-side shard prep.  Returns in_maps for the 8 cores."""
    kc_n = nk // P
    # bkv_V folds into bp exactly: softmax rows sum to 1, so the +bkvV on V
    # adds bkvV @ Wp to every output row.  bb / bkv_K cancel in softmax.
    bp_eff = (np.asarray(bp, dtype=np.float64)
              + np.asarray(bkv[H * D:], dtype=np.float64)
              @ np.asarray(Wp, dtype=np.float64)).astype(np.float32)

    def halves(a2d):        # [2*P rows, m] -> [P, 2, m] -> [P, 2*m]
        r = np.asarray(a2d)
        return (r.reshape(2, P, -1).transpose(1, 0, 2)
                .reshape(P, -1).astype(NP_BF))

    wb_bf = np.ascontiguousarray(Wb).astype(NP_BF)
    pk_shared = np.empty((P, PK_END), dtype=NP_BF)
    pk_shared[:, PK_WB:PK_BQ] = wb_bf
    pk_shared[:, PK_BQ:PK_WQ] = (np.asarray(bq).reshape(2, P)
                                 .transpose(1, 0).astype(NP_BF))
    pk_shared[:, PK_WQ:PK_XQ] = halves(Wq)
    pk_shared[:, PK_WKV:PK_XKV] = halves(Wkv)
    pk_shared[:, PK_XKV:PK_WP] = halves(x_kv[0].T)
    pk_shared[:, PK_WP:PK_END] = halves(Wp)
    in_maps = []
    for i in range(NCORES):
        qs = i * NQC
        zi = z[0, qs:qs + NQC]                           # [q, k, c]
        # -> [s=(kc,qh), c, q64, k] with q = qh*64 + t, k = kc*128 + k
        zi = (zi.reshape(2, QH, kc_n, P, BD)
              .transpose(2, 0, 4, 1, 3)                  # [kc, qh, c, t, k]
              .reshape(kc_n * 2, BD, QH, P))
        pk = pk_shared.copy()
        pk[:, PK_XQ:PK_WKV] = halves(x_q[0, qs:qs + NQC].T)
        in_maps.append(dict(
            zT=np.ascontiguousarray(zi).astype(NP_F8),
            wbT=wb_bf,
            packB=pk,
            bp=bp_eff,
        ))
    return in_maps


_NC_CACHE = {}


def kernel(x_q, x_kv, z, Wq, bq, Wkv, bkv, Wb, bb, Wp, bp):
    key = "full"
    if key not in _NC_CACHE:
        _NC_CACHE[key] = build_program()
    nc = _NC_CACHE[key]
    in_maps = prep_inputs(x_q, x_kv, z, Wq, bq, Wkv, bkv, Wb, bb, Wp, bp)
    res = run_bass_kernel_spmd(nc, in_maps, list(range(NCORES)))
    out = np.empty((1, NQ, CQ), dtype=np.float32)
    for i in range(NCORES):
        out[0, i * NQC:(i + 1) * NQC, :] = res.results[i]["y"]
    return out


# revision 36
# speedup vs baseline: 1.1252x; 1.0019x over previous
"""BiasAttention TRN2 kernel — q-sharded across 8 NeuronCores.

Each core owns a block of 128 queries and computes the full attention for
them (all 8 heads, all 1024 keys), including the z-bias projection, with no
collectives.  The whole attention is computed TRANSPOSED (S^T[k,q],
bias^T[k,q]) so the exp'd scores feed the PV matmul directly with no PE
transposes.  Host-side prep lays z out per core as [kc, c, q, k] (contract
dim c on partitions, one query's [c,k] tile per stationary) and casts z to
fp8e4m3 — halving the dominant HBM stream.  Exact-math folds: bb and the
K-side bkv bias are constant along the softmax axis (cancel), and the
V-side bkv bias folds into bp on host (softmax rows sum to 1).

z streams as 1MB [c, 64q, 128k] transfers split across BOTH HWDGE rings so
the per-transfer completion latency overlaps and the stream runs near the
~360GB/s HBM-per-core limit.  All small consts ride in ONE per-partition-
contiguous packed transfer at the head of the scalar ring (separate
rearranged transfers produce 512B descriptors that crawl when fair-shared
against the z stream).  S^T / V builds are hoisted into the early z-ramp
window; the epilogue runs in bf16 (fp32 matmuls are 4 cyc/row).
"""

import sys

if "/opt/trn_rl_repo" not in sys.path:
    sys.path.insert(0, "/opt/trn_rl_repo")

import ml_dtypes
import numpy as np

import concourse.bass as bass
import concourse.mybir as mybir
from concourse import bacc
from concourse.bass_utils import run_bass_kernel_spmd
from concourse.masks import make_identity
from concourse.tile import TileContext

P = 128          # partitions
H = 8            # heads
D = 32           # head dim
CQ = 256         # q channels
CKV = 256        # kv channels
BD = 128         # bias (z) channels
NQ = 1024        # total queries
NCORES = 8
NQC = NQ // NCORES   # queries per core = 128
SCALE = D ** (-0.5)

QH = 64          # queries per z half-step (1MB fp8 transfer)
FP = mybir.dt.float32
BF = mybir.dt.bfloat16
F8 = mybir.dt.float8e4
NP_BF = ml_dtypes.bfloat16
NP_F8 = ml_dtypes.float8_e4m3

# bf16 const pack layout (elements per partition): wb, bq, Wq, xqT, Wkv,
# xkvT, Wp — all [p, ...] with the 2 c/hd-halves as the middle dim.
PK_WB = 0
PK_BQ = PK_WB + H
PK_WQ = PK_BQ + 2
PK_XQ = PK_WQ + 2 * CQ
PK_WKV = PK_XQ + 2 * NQC
PK_XKV = PK_WKV + 2 * 2 * H * D
PK_WP = PK_XKV + 2 * 1024
PK_END = PK_WP + 2 * CQ

# All z rides the sync HWDGE ring: only 8 HWDGE semaphore lanes exist
# (shared across both rings in scheduler order), so splitting z across
# rings couples their pacing and lets the Tile scheduler push the second
# ring's triggers behind the loop's exps.  One consumption-ordered ring
# sustains ~400GB/s by itself; the scalar ring carries only the const
# pack, and the kT32/qT32 shuffles use the gpsimd SWDGE (separate
# semaphore pool).
SCALAR_S = ()


def build_program(nk=1024):
    kc_n = nk // P            # k-chunks of 128
    ns = kc_n * 2             # z half-steps (64 queries x one k-chunk each)
    add = mybir.AluOpType.add
    mult = mybir.AluOpType.mult

    nc = bacc.Bacc("TRN2", target_bir_lowering=False, debug=False,
                   num_devices=NCORES)

    # ---- I/O ----
    zT = nc.dram_tensor("zT", [ns, BD, QH, P], F8, kind="ExternalInput")
    wbT = nc.dram_tensor("wbT", [BD, H], BF, kind="ExternalInput")
    packB = nc.dram_tensor("packB", [P, PK_END], BF, kind="ExternalInput")
    bp = nc.dram_tensor("bp", [CQ], FP, kind="ExternalInput")
    y = nc.dram_tensor("y", [NQC, CQ], FP, kind="ExternalOutput")

    with TileContext(nc) as tc:
        with (
            tc.tile_pool(name="const", bufs=1) as const,
            tc.tile_pool(name="zpool", bufs=15) as zpool,
            tc.tile_pool(name="epool", bufs=6) as epool,
            tc.tile_pool(name="xpool", bufs=6) as xpool,
            tc.tile_pool(name="stv_ps", bufs=2, space="PSUM") as stv_ps,
            tc.tile_pool(name="b_ps", bufs=5, space="PSUM") as b_psp,
            tc.tile_pool(name="o_ps", bufs=1, space="PSUM") as o_psp,
        ):
            # ---- const pack leads the scalar ring: one contiguous 8KB-per-
            # partition transfer (big descriptors survive fair-sharing with
            # the z stream).
            # wb leads as its own 2KB transfer: the very first bias matmuls
            # gate only on it plus the first z quarter.
            wb_sb = const.tile([P, H], BF)
            nc.scalar.dma_start(wb_sb, wbT[:])
            pk = const.tile([P, PK_END], BF)
            nc.scalar.dma_start(pk, packB[:])
            bq_sb = pk[:, PK_BQ:PK_WQ]
            wq_sb = pk[:, PK_WQ:PK_XQ].rearrange("p (o m) -> p o m", o=2)
            xqT_sb = pk[:, PK_XQ:PK_WKV].rearrange("p (o q) -> p o q", o=2)
            wkv_sb = pk[:, PK_WKV:PK_XKV].rearrange("p (o m) -> p o m", o=2)
            xkvT_sb = pk[:, PK_XKV:PK_WP].rearrange("p (o k) -> p o k", o=2)
            wp_sb = pk[:, PK_WP:PK_END].rearrange("p (o m) -> p o m", o=2)

            # ---- z stream: sync ring is pure z; the first half is split
            # into two 512KB transfers so the very first bias matmuls can
            # start sooner.
            zs = {}
            for s in range(ns):
                z_sb = zpool.tile([P, QH, P], F8, tag="z", name=f"z{s}")
                if s not in SCALAR_S:
                    if s == 0:
                        # 4x 256KB so the very first bias matmuls start early
                        for qq in range(4):
                            nc.sync.dma_start(z_sb[:, qq * 16:(qq + 1) * 16, :],
                                              zT[s, :, qq * 16:(qq + 1) * 16, :])
                    else:
                        nc.sync.dma_start(z_sb, zT[s])
                zs[s] = z_sb

            # bf16 cast during the SWDGE transfer (epilogue-only, tiny)
            bp_sb = const.tile([1, CQ], BF)
            nc.gpsimd.dma_start(bp_sb, bp[None, :])
            ident = const.tile([P, P], BF)
            make_identity(nc, ident)
            ones_row = const.tile([1, P], BF)
            nc.vector.memset(ones_row, 1.0)

            # V augmented with a ones column per head: [k, kc, h, D+1]
            vaug_sb = const.tile([P, kc_n, H, D + 1], BF)
            nc.vector.memset(vaug_sb, 1.0)

            # ---- bias^T z matmuls: per query a [c,k] stationary and an
            # 8-col Wb moving op into b_ps [k, (q64 h8)].
            bias_done = {}

            def bias_mms(b_ps, s, t0, t1):
                z_sb = zs[s]
                for t in range(t0, t1):
                    nc.tensor.matmul(b_ps[:, t * H:(t + 1) * H],
                                     lhsT=z_sb[:, t, :], rhs=wb_sb,
                                     start=(t == 0), stop=(t == QH - 1))

            def bias_half(s):
                if s in bias_done:
                    return bias_done[s]
                b_ps = b_psp.tile([P, QH * H], FP, tag="b")
                bias_mms(b_ps, s, 0, QH)
                bias_done[s] = b_ps
                return b_ps

            # hoisted startup: interleave the first z halves with the
            # projections in z-arrival order.
            b_ps0 = b_psp.tile([P, QH * H], FP, tag="b")
            bias_done[0] = b_ps0
            bias_mms(b_ps0, 0, 0, 16)        # s0 first quarter
            bias_mms(b_ps0, 0, 16, 32)

            # Q^T [(h d), q] with (x + bq) * SCALE folded in, stored bf16
            bq_f32 = const.tile([P, 2], FP)
            nc.vector.tensor_copy(bq_f32, bq_sb)
            qT_sb = const.tile([P, 2, NQC], BF)
            for m in range(2):
                ps = stv_ps.tile([P, 512], FP, tag="stv")
                for c in range(2):
                    nc.tensor.matmul(ps[:, :NQC],
                                     lhsT=wq_sb[:, c, m * P:(m + 1) * P],
                                     rhs=xqT_sb[:, c, :],
                                     start=(c == 0), stop=(c == 1))
                nc.vector.tensor_scalar(qT_sb[:, m, :], ps[:, :NQC],
                                        bq_f32[:, m:m + 1], SCALE, add, mult)

            bias_mms(b_ps0, 0, 32, QH)       # s0b

            # K^T [(h d), k] — bkv_K is softmax-invariant, dropped
            kT_sb = const.tile([P, 2, nk], BF)
            for m in range(2):
                for nh in range((nk + 511) // 512):
                    nn_ = min(512, nk - nh * 512)
                    ps = stv_ps.tile([P, 512], FP, tag="stv")
                    for c in range(2):
                        nc.tensor.matmul(ps[:, :nn_],
                                         lhsT=wkv_sb[:, c, m * P:(m + 1) * P],
                                         rhs=xkvT_sb[:, c, nh * 512:nh * 512 + nn_],
                                         start=(c == 0), stop=(c == 1))
                    if (m * 2 + nh) % 2 == 0:
                        nc.scalar.activation(
                            kT_sb[:, m, nh * 512:nh * 512 + nn_], ps[:, :nn_],
                            mybir.ActivationFunctionType.Copy)
                    else:
                        nc.vector.tensor_copy(
                            kT_sb[:, m, nh * 512:nh * 512 + nn_], ps[:, :nn_])

            bias_half(1)
            bias_half(2)

            # qT/kT reshuffled so every head's d-dim sits on partitions
            # 0-31: the S^T matmuls then run at default tile position and
            # can share psum banks at different column offsets (the same
            # pattern the z matmuls use) — tile-positioned matmuls sharing
            # a bank wedge the device.  kT32 rides the scalar ring between
            # the early and late z halves; qT32 rides gpsimd.
            qT32 = const.tile([32, H, NQC], BF)
            kT32 = const.tile([32, H, nk], BF)
            for hq in range(4):
                nc.gpsimd.dma_start(qT32[:, hq::4, :],
                                    qT_sb[hq * 32:(hq + 1) * 32, :, :])
            for m in range(2):
                for hq in range(4):
                    nc.gpsimd.dma_start(kT32[:, m * 4 + hq, :],
                                        kT_sb[hq * 32:(hq + 1) * 32, m, :])

            # S^T[k, kc, h, q]: per (kc, head) a [k=128, q=128] matmul;
            # 4 heads batch into one psum tile with a single copy out.
            sT_sb = const.tile([P, kc_n, H, NQC], BF)

            def st_build(kc):
                for hg in range(2):
                    ps = stv_ps.tile([P, 512], FP, tag="stv", name="st_ps")
                    for hl in range(4):
                        h = hg * 4 + hl
                        nc.tensor.matmul(ps[:, hl * P:(hl + 1) * P],
                                         lhsT=kT32[:, h, kc * P:(kc + 1) * P],
                                         rhs=qT32[:, h, :],
                                         start=True, stop=True)
                    nc.vector.tensor_copy(
                        sT_sb[:, kc, hg * 4:(hg + 1) * 4, :],
                        ps.rearrange("p (hl q) -> p hl q", hl=4))

            # V [k, (h d)] (bias folded into bp on host) into vaug
            def v_build(kc):
                ps = stv_ps.tile([P, 512], FP, tag="stv", name="v_ps")
                for c in range(2):
                    nc.tensor.matmul(ps[:, :H * D],
                                     lhsT=xkvT_sb[:, c, kc * P:(kc + 1) * P],
                                     rhs=wkv_sb[:, c, H * D:2 * H * D],
                                     start=(c == 0), stop=(c == 1))
                nc.vector.tensor_copy(
                    vaug_sb[:, kc, :, 0:D],
                    ps[:, :H * D].rearrange("p (h d) -> p h d", h=H))

            # all S^T / V builds run up front, filling the early window
            # while the z stream is still ramping.
            for kc in range(kc_n):
                v_build(kc)
            for kc in range(kc_n):
                st_build(kc)

            # ---- main loop over k-chunks: pure bias+PV on the PE, with
            # the add->exp chain two chunks behind the z matmuls.
            o_ps = o_psp.tile([P, H * (D + 1)], FP)   # [q, h*(D+1)]
            xTs = {}

            def pv_emit(kc):
                xT_sb = xTs.pop(kc)
                for h in range(H):
                    # o_ps lives in one bank: open the accumulation group
                    # on the first matmul only, close on the last.
                    nc.tensor.matmul(
                        o_ps[:, h * (D + 1):(h + 1) * (D + 1)],
                        lhsT=xT_sb[:, h, :], rhs=vaug_sb[:, kc, h, :],
                        start=(kc == 0 and h == 0),
                        stop=(kc == kc_n - 1 and h == H - 1))

            for kc in range(kc_n):
                bps = [bias_half(2 * kc), bias_half(2 * kc + 1)]
                # e^T holds the chunk's logits [k, h, q] (bf16)
                e_sb = epool.tile([P, H, NQC], BF, tag="e")
                for hf in range(2):
                    bv = bps[hf].rearrange("p (q h) -> p h q", h=H)
                    qs = slice(hf * QH, (hf + 1) * QH)
                    for hg in range(2):
                        hh = slice(hg * 4, (hg + 1) * 4)
                        nc.vector.tensor_tensor(
                            e_sb[:, hh, qs], sT_sb[:, kc, hh, qs],
                            bv[:, hh, :], add)
                # lag 4: the bias stream runs at the b_ps ring limit (~2.5
                # chunks ahead of the adds), so exp(kc-3) lands exactly when
                # pv(kc-3) would need it — one more chunk of lag removes the
                # ~2.2us PE stall per chunk.
                if kc - 4 in xTs:
                    pv_emit(kc - 4)
                xT_sb = xpool.tile([P, H, NQC], BF, tag="x")
                xTs[kc] = xT_sb
                for hg in range(2):          # head groups of 4
                    hh = slice(hg * 4, (hg + 1) * 4)
                    nc.scalar.activation(xT_sb[:, hh, :], e_sb[:, hh, :],
                                         mybir.ActivationFunctionType.Exp)
            for kc in range(kc_n - 4, kc_n):
                pv_emit(kc)

            # ---- epilogue: normalize, transpose, output projection, all
            # in bf16 (fp32 matmuls run at 1/4 rate).
            recip_sb = const.tile([P, H], FP)
            nc.vector.reciprocal(
                recip_sb, o_ps.rearrange("p (h e) -> p h e", h=H)[:, :, D])
            o_sb = const.tile([P, 2, P], BF)     # [q, half, (h d)%128]
            ov = o_ps.rearrange("p (h e) -> p h e", h=H)
            rv = bass.AP(tensor=recip_sb.tensor, offset=recip_sb.offset,
                         ap=[list(recip_sb.ap[0])]
                         + [[recip_sb.ap[1][0], 4], [0, D]])
            for half in range(2):
                nc.vector.tensor_tensor(
                    o_sb[:, half, :].rearrange("p (h d) -> p h d", h=4),
                    ov[:, half * 4:(half + 1) * 4, 0:D],
                    bass.AP(tensor=rv.tensor,
                            offset=rv.offset + half * 4 * recip_sb.ap[1][0],
                            ap=rv.ap), mult)
            oT_sb = const.tile([P, 2, P], BF)
            for m in range(2):
                t_full = stv_ps.tile([P, 1024], BF, tag="stv", name="t_full")
                t_ps = t_full[:, :P]
                nc.tensor.transpose(t_ps, o_sb[:, m, :], ident)
                nc.vector.tensor_copy(oT_sb[:, m, :], t_ps)
            ps = stv_ps.tile([P, 512], FP, tag="stv")
            for m in range(2):
                nc.tensor.matmul(ps[:, :CQ], lhsT=oT_sb[:, m, :],
                                 rhs=wp_sb[:, m, :], start=(m == 0), stop=False)
            nc.tensor.matmul(ps[:, :CQ], lhsT=ones_row, rhs=bp_sb,
                             start=False, stop=True)
            y_sb = const.tile([P, CQ], FP)
            nc.vector.tensor_copy(y_sb, ps[:, :CQ])
            nc.sync.dma_start(y[:], y_sb)

    nc.compile()
    return nc


def prep_inputs(x_q, x_kv, z, Wq, bq, Wkv, bkv, Wb, bb, Wp, bp, nk=1024):
    """Host-side shard prep.  Returns in_maps for the 8 cores."""
    kc_n = nk // P
    # bkv_V folds into bp exactly: softmax rows sum to 1, so the +bkvV on V
    # adds bkvV @ Wp to every output row.  bb / bkv_K cancel in softmax.
    bp_eff = (np.asarray(bp, dtype=np.float64)
              + np.asarray(bkv[H * D:], dtype=np.float64)
              @ np.asarray(Wp, dtype=np.float64)).astype(np.float32)

    def halves(a2d):        # [2*P rows, m] -> [P, 2, m] -> [P, 2*m]
        r = np.asarray(a2d)
        return (r.reshape(2, P, -1).transpose(1, 0, 2)
                .reshape(P, -1).astype(NP_BF))

    wb_bf = np.ascontiguousarray(Wb).astype(NP_BF)
    pk_shared = np.empty((P, PK_END), dtype=NP_BF)
    pk_shared[:, PK_WB:PK_BQ] = wb_bf
    pk_shared[:, PK_BQ:PK_WQ] = (np.asarray(bq).reshape(2, P)
                                 .transpose(1, 0).astype(NP_BF))
    pk_shared[:, PK_WQ:PK_XQ] = halves(Wq)
    pk_shared[:, PK_WKV:PK_XKV] = halves(Wkv)
    pk_shared[:, PK_XKV:PK_WP] = halves(x_kv[0].T)
    pk_shared[:, PK_WP:PK_END] = halves(Wp)
    in_maps = []
    for i in range(NCORES):
        qs = i * NQC
        zi = z[0, qs:qs + NQC]                           # [q, k, c]
        # -> [s=(kc,qh), c, q64, k] with q = qh*64 + t, k = kc*128 + k
        zi = (zi.reshape(2, QH, kc_n, P, BD)
              .transpose(2, 0, 4, 1, 3)                  # [kc, qh, c, t, k]
              .reshape(kc_n * 2, BD, QH, P))
        pk = pk_shared.copy()
        pk[:, PK_XQ:PK_WKV] = halves(x_q[0, qs:qs + NQC].T)
        in_maps.append(dict(
            zT=np.ascontiguousarray(zi).astype(NP_F8),
            wbT=wb_bf,
            packB=pk,
            bp=bp_eff,
        ))
    return in_maps


_NC_CACHE = {}


def kernel(x_q, x_kv, z, Wq, bq, Wkv, bkv, Wb, bb, Wp, bp):
    key = "full"
    if key not in _NC_CACHE:
        _NC_CACHE[key] = build_program()
    nc = _NC_CACHE[key]
    in_maps = prep_inputs(x_q, x_kv, z, Wq, bq, Wkv, bkv, Wb, bb, Wp, bp)
    res = run_bass_kernel_spmd(nc, in_maps, list(range(NCORES)))
    out = np.empty((1, NQ, CQ), dtype=np.float32)
    for i in range(NCORES):
        out[0, i * NQC:(i + 1) * NQC, :] = res.results[i]["y"]
    return out
